# revision 2
# baseline (speedup 1.0000x reference)
"""Trainium2 Bass kernel for nn_Attention_19739669692939 (sparse_attention).

Reference computation (shapes: L=1024, B=64, C=1024, D=512, E=512):
    Wa_e = W_attn[:, :C]        # [E, C]
    Wa_s = W_attn[:, C:]        # [E, D]
    pre  = enc_output @ Wa_e.T + s @ Wa_s.T     # [L, B, E] (s broadcast over L)
    engry = tanh(pre)
    att[b, l] = engry[l, b, :] @ W_v[0, :]
    out = softmax(att, axis=-1)                 # [B, 1024]

Distribution: pure data-parallel over batch. Core i handles batches
[8i, 8i+8); no collectives.

v2 design: ALL data-layout work happens on the host before staging to
HBM, so the device runs matmuls only:
- enc is pre-cast on host (fp8e4m3 for c<768, bf16 for c>=768) and
  pre-arranged in the exact SBUF image the PE consumes: the fp8 half in
  DoubleRow k-pair-interleaved [p=c-pair, (pc, l, kt)] layout, the bf16
  quarter in [p=c, (cb, l)] layout. Device DMAs are plain contiguous
  [128, N] block loads -- zero PE transposes, zero DVE copies, and HBM
  traffic drops from 32 MB to ~10.5 MB per core.
- W_attn is pre-scaled (x256, halves fp8 subnormal loss; the tanh's
  scale=1/256 undoes it), pre-cast, and pre-transposed on host into the
  DR weight layout [p, (pc, kt, e)] plus the bf16 [p, (cb, e)] blocks.
- bias[e,b] = Wa_s @ s[b].T is computed exactly on host (f64) -- the
  d-blocks never ship to the device.
- The masked-W_v tiles (column b holds W_v, zeros elsewhere) are built
  on host; the W_v contraction lands in PSUM row b for batch b, with
  the four e-blocks col-packed at PSUM partitions {0,32,64,96}+[0,8).
- The host gather sums the four col-group blocks and applies the
  softmax (cheap [8,1024] numpy), as before.

Device steady state per (lc, b) unit: 8 bf16 MMs + 12 fp8 DoubleRow MMs
(K=256 each) + 4 masked-W_v MMs, all N=512, with the deferred-wv MMs
used as LDWEIGHTS-hiding filler inside the next unit's MM stream; tanh
(+exact bias) on ACT in parallel. Everything is prefetched: all 32 enc
block DMAs are issued up front (SWDGE ring carries the fp8 halves,
HWDGE the consts + bf16 halves), so after a ~3us ramp the PE never
waits. A short dependency-free garbage-transpose burst covers the ramp
so the PE p-state is hot when the first real data lands.
"""

import numpy as np
import ml_dtypes

import concourse.bass as bass
import concourse.mybir as mybir
from concourse import bacc
from concourse.bass_utils import run_bass_kernel_spmd
from concourse.tile import TileContext

F32 = mybir.dt.float32
BF16 = mybir.dt.bfloat16
FP8 = mybir.dt.float8e4
AF = mybir.ActivationFunctionType
F8NP = ml_dtypes.float8_e4m3
BF16NP = ml_dtypes.bfloat16

L = 1024          # enc length
B = 64            # global batch
BL = 8            # batch per core
C = 1024          # enc feature dim (2*enc_hid)
D = 512           # dec feature dim
E = 512           # engry dim
NCORES = 8

NEB = E // 128    # 4 e-blocks
LCH = 512         # l-chunk processed per unit
NLC = L // LCH    # 2 chunks

# fp8 split: c < C8 runs in fp8e4 DoubleRow (2 c-blocks per matmul),
# c in [C8, C) stays bf16. W is pre-scaled by WSCALE before the fp8
# cast; the tanh activation's scale undoes it.
NC8 = 6           # fp8 c-blocks
NC16 = C // 128 - NC8  # bf16 c-blocks (2)
WSCALE = 256.0
C8 = NC8 * 128    # fp8 c-range (768)
NPC = NC8 // 2    # 256-c pair-chunks (3)


def build_nc():
    nc = bacc.Bacc("TRN2", target_bir_lowering=False, debug=False)

    enc8 = nc.dram_tensor("enc8", [NLC, BL, 128, NPC * 2 * LCH], FP8,
                          kind="ExternalInput").ap()
    enc16 = nc.dram_tensor("enc16", [NLC, BL, 128, NC16 * LCH], BF16,
                           kind="ExternalInput").ap()
    waT8p_d = nc.dram_tensor("waT8p", [128, NPC * 2 * E], FP8,
                             kind="ExternalInput").ap()
    waT16_d = nc.dram_tensor("waT16", [128, NC16 * E], BF16,
                             kind="ExternalInput").ap()
    bias_d = nc.dram_tensor("bias", [128, NEB * BL], F32,
                            kind="ExternalInput").ap()
    wvm_d = nc.dram_tensor("wv_mask", [128, NEB * BL * BL], BF16,
                           kind="ExternalInput").ap()
    # Blocked attention logits: per l-chunk, the four eb col-group blocks
    # live at PSUM partition rows {0,32,64,96}+[0,8). The host sums the
    # blocks and applies the softmax.
    out = nc.dram_tensor("out", [NLC, 128, LCH], F32, kind="ExternalOutput").ap()

    with TileContext(nc) as tc:
        with (
            tc.tile_pool(name="consts", bufs=1) as consts,
            tc.tile_pool(name="e8p", bufs=NLC * BL) as e8_pool,
            tc.tile_pool(name="e16p", bufs=NLC * BL) as e16_pool,
            tc.tile_pool(name="engry", bufs=2) as engry_pool,
            tc.tile_pool(name="pre", bufs=4, space="PSUM") as pre_pool,
            tc.tile_pool(name="att", bufs=2, space="PSUM") as att_pool,
            tc.tile_pool(name="warm", bufs=1, space="PSUM") as warm_pool,
        ):
            # p-state warmup: dependency-free garbage transposes keep the
            # PE pipe hot while the first DMAs land (output never read).
            garbage = consts.tile([128, 128], BF16, tag="garbage")
            nc.vector.memset(garbage[:], 0.0)
            warm_ps = warm_pool.tile([128, 512], BF16, tag="warm")
            for i in range(48):
                nc.tensor.transpose(
                    warm_ps[:, (i % 4) * 128:(i % 4) * 128 + 128],
                    garbage[:], garbage[:])

            # Consts ride HWDGE; the first unit's bf16 half goes right
            # after waT16 so the opening bf16 MM isn't starved. The SWDGE
            # ring carries only the fp8 enc halves, in unit order.
            waT8p = consts.tile([128, NPC * 2 * E], FP8, tag="waT8p")
            nc.sync.dma_start(out=waT8p[:], in_=waT8p_d[:, :])
            waT16 = consts.tile([128, NC16 * E], BF16, tag="waT16")
            nc.sync.dma_start(out=waT16[:], in_=waT16_d[:, :])

            e8_t, e16_t = {}, {}

            def fetch8(lc, b):
                t8 = e8_pool.tile([128, NPC * 2 * LCH], FP8, tag="e8",
                                  name=f"e8_{lc}_{b}")
                nc.gpsimd.dma_start(out=t8[:], in_=enc8[lc, b])
                e8_t[(lc, b)] = t8

            def fetch16(lc, b):
                t16 = e16_pool.tile([128, NC16 * LCH], BF16, tag="e16",
                                    name=f"e16_{lc}_{b}")
                nc.sync.dma_start(out=t16[:], in_=enc16[lc, b])
                e16_t[(lc, b)] = t16

            fetch8(0, 0)
            fetch16(0, 0)

            bias_sbuf = consts.tile([128, NEB * BL], F32, tag="bias")
            nc.sync.dma_start(out=bias_sbuf[:], in_=bias_d[:, :])
            wv_mask = consts.tile([128, NEB * BL * BL], BF16, tag="wvm")
            nc.sync.dma_start(out=wv_mask[:], in_=wvm_d[:, :])

            for lc in range(NLC):
                for b in range(BL):
                    if (lc, b) != (0, 0):
                        fetch8(lc, b)
                        fetch16(lc, b)

            waT8v = waT8p.rearrange("p (pc two e) -> p pc two e",
                                    pc=NPC, two=2)

            # ---------------- main loop ----------------
            # att accumulation: eb's result lands in PSUM partitions
            # [32eb, 32eb+8), accumulated over b. Each b's wv matmuls are
            # DEFERRED into the next b's MM stream: one per eb-block,
            # placed right before the last DR matmul so the 256-col DR
            # LDWEIGHTS hides under the wv matmul's 512-row stream.
            for lc in range(NLC):
                att_ps = att_pool.tile([128, LCH], F32, tag="att")

                def emit_wv(b, engries, eb):
                    nc.tensor.matmul(
                        att_ps[32 * eb:32 * eb + BL, :],
                        lhsT=wv_mask[:, eb * BL * BL + b * BL:
                                     eb * BL * BL + (b + 1) * BL],
                        rhs=engries[eb][:],
                        start=(b == 0),
                        stop=(b == BL - 1),
                        tile_position=(0, 32 * eb),
                    )

                pending = None
                for b in range(BL):
                    e8v = e8_t[(lc, b)].rearrange(
                        "p (pc l two) -> p pc two l", pc=NPC, two=2)
                    e16 = e16_t[(lc, b)]
                    engries = []
                    for eb in range(NEB):
                        pre = pre_pool.tile([128, LCH], F32, tag="pre")
                        nc.tensor.matmul(
                            pre[:],
                            lhsT=waT16[:, 0 * E + eb * 128:0 * E + (eb + 1) * 128],
                            rhs=e16[:, 0:LCH],
                            start=True, stop=False,
                        )
                        nc.tensor.matmul(
                            pre[:],
                            lhsT=waT8v[:, 0, :, eb * 128:(eb + 1) * 128],
                            rhs=e8v[:, 0],
                            start=False, stop=False,
                            perf_mode=mybir.MatmulPerfMode.DoubleRow,
                        )
                        nc.tensor.matmul(
                            pre[:],
                            lhsT=waT16[:, 1 * E + eb * 128:1 * E + (eb + 1) * 128],
                            rhs=e16[:, LCH:2 * LCH],
                            start=False, stop=False,
                        )
                        nc.tensor.matmul(
                            pre[:],
                            lhsT=waT8v[:, 1, :, eb * 128:(eb + 1) * 128],
                            rhs=e8v[:, 1],
                            start=False, stop=False,
                            perf_mode=mybir.MatmulPerfMode.DoubleRow,
                        )
                        if pending is not None:
                            emit_wv(pending[0], pending[1], eb)
                        nc.tensor.matmul(
                            pre[:],
                            lhsT=waT8v[:, 2, :, eb * 128:(eb + 1) * 128],
                            rhs=e8v[:, 2],
                            start=False, stop=True,
                            perf_mode=mybir.MatmulPerfMode.DoubleRow,
                        )
                        engry = engry_pool.tile([128, LCH], BF16,
                                                tag=f"engry{eb}",
                                                name=f"engry{eb}_{lc}_{b}")
                        nc.scalar.activation(
                            engry[:], pre[:], AF.Tanh,
                            bias=bias_sbuf[:, eb * BL + b: eb * BL + b + 1],
                            scale=1.0 / WSCALE,
                        )
                        engries.append(engry)
                    pending = (b, engries)
                # flush the last b's wv matmuls, then ship the blocked
                # logits straight from PSUM.
                for eb in range(NEB):
                    emit_wv(pending[0], pending[1], eb)
                att_cp = consts.tile([128, LCH], F32, tag="att_cp",
                                     name=f"att_cp{lc}")
                nc.vector.tensor_copy(att_cp[:], att_ps[:])
                nc.sync.dma_start(out=out[lc], in_=att_cp[:])

    nc.compile()
    return nc


_NC_CACHE = None


def _get_nc():
    global _NC_CACHE
    if _NC_CACHE is None:
        _NC_CACHE = build_nc()
    return _NC_CACHE


def make_in_maps(enc_output, s, W_attn, W_v):
    enc = np.asarray(enc_output, dtype=np.float32)   # [L, B, C]
    s = np.asarray(s, dtype=np.float32)              # [1, B, D]
    W = np.asarray(W_attn, dtype=np.float32)         # [E, C+D]
    wv = np.asarray(W_v, dtype=np.float32)           # [1, E]

    # enc fp8 half -> DoubleRow k-pair image [lc, b, p, (pc, l, kt)]
    # with c = pc*256 + 2p + kt.
    e8 = enc[:, :, :C8].astype(F8NP)                 # [L, B, C8]
    e8 = e8.view(np.uint8).reshape(NLC, LCH, B, NPC, 128, 2)
    e8 = np.ascontiguousarray(e8.transpose(0, 2, 4, 3, 1, 5))
    e8 = e8.reshape(NLC, B, 128, NPC * LCH * 2).view(F8NP)

    # enc bf16 quarter -> [lc, b, p, (cb, l)] with c = C8 + cb*128 + p.
    e16 = enc[:, :, C8:].astype(BF16NP)              # [L, B, C-C8]
    e16 = e16.reshape(NLC, LCH, B, NC16, 128)
    e16 = np.ascontiguousarray(e16.transpose(0, 2, 4, 3, 1))
    e16 = e16.reshape(NLC, B, 128, NC16 * LCH)

    # DR weights [p, (pc, kt, e)] = fp8(WSCALE * W[e, pc*256 + 2p + kt])
    w8 = (W[:, :C8] * WSCALE).astype(F8NP)           # [E, C8]
    w8 = w8.reshape(E, NPC, 128, 2)                  # [e, pc, p, kt]
    waT8p = np.ascontiguousarray(w8.transpose(2, 1, 3, 0)).reshape(
        128, NPC * 2 * E)

    # bf16 weights [p, (cb, e)] = bf16(WSCALE * W[e, C8 + cb*128 + p])
    w16 = (W[:, C8:C] * WSCALE).astype(BF16NP)       # [E, NC16*128]
    w16 = w16.reshape(E, NC16, 128)
    waT16 = np.ascontiguousarray(w16.transpose(2, 1, 0)).reshape(
        128, NC16 * E)

    # exact bias[e, b] = Wa_s @ s[b].T in f64
    bias_full = np.einsum(
        'ed,bd->eb', W[:, C:].astype(np.float64),
        s[0].astype(np.float64)).astype(np.float32)  # [E, B]

    # masked W_v: [p, (eb, b, j)], column j==b holds wv[eb*128+p]
    wvm = np.zeros((128, NEB, BL, BL), np.float32)
    wvr = wv[0].reshape(NEB, 128)                    # [eb, p]
    for b in range(BL):
        wvm[:, :, b, b] = wvr.T
    wv_mask = np.ascontiguousarray(
        wvm.astype(BF16NP).reshape(128, NEB * BL * BL))

    in_maps = []
    for i in range(NCORES):
        bias = np.ascontiguousarray(
            bias_full[:, i * BL:(i + 1) * BL].reshape(NEB, 128, BL)
            .transpose(1, 0, 2).reshape(128, NEB * BL))
        in_maps.append({
            "enc8": np.ascontiguousarray(e8[:, i * BL:(i + 1) * BL]),
            "enc16": np.ascontiguousarray(e16[:, i * BL:(i + 1) * BL]),
            "waT8p": waT8p,
            "waT16": waT16,
            "bias": bias,
            "wv_mask": wv_mask,
        })
    return in_maps


def kernel(enc_output, s, W_attn, W_v):
    nc = _get_nc()
    in_maps = make_in_maps(enc_output, s, W_attn, W_v)
    res = run_bass_kernel_spmd(nc, in_maps, core_ids=list(range(NCORES)))
    outs = []
    for i in range(NCORES):
        blk = res.results[i]["out"]  # [NLC, 128, LCH] blocked logits
        att = sum(blk[:, 32 * g:32 * g + BL, :] for g in range(4))  # [NLC, BL, LCH]
        att = np.concatenate([att[lc] for lc in range(NLC)], axis=1)  # [BL, L]
        m = att.max(axis=1, keepdims=True)
        e = np.exp(att - m)
        outs.append((e / e.sum(axis=1, keepdims=True)).astype(np.float32))
    return np.concatenate(outs, axis=0)


# revision 5
# speedup vs baseline: 1.2018x; 1.2018x over previous
"""Trainium2 Bass kernel for nn_Attention_19739669692939 (sparse_attention).

Reference computation (shapes: L=1024, B=64, C=1024, D=512, E=512):
    Wa_e = W_attn[:, :C]        # [E, C]
    Wa_s = W_attn[:, C:]        # [E, D]
    pre  = enc_output @ Wa_e.T + s @ Wa_s.T     # [L, B, E] (s broadcast over L)
    engry = tanh(pre)
    att[b, l] = engry[l, b, :] @ W_v[0, :]
    out = softmax(att, axis=-1)                 # [B, 1024]

Distribution: pure data-parallel over batch. Core i handles batches
[8i, 8i+8); no collectives.

v2 design: ALL data-layout work happens on the host before staging to
HBM, so the device runs matmuls only:
- enc is pre-cast on host (fp8e4m3 for c<768, bf16 for c>=768) and
  pre-arranged in the exact SBUF image the PE consumes: the fp8 half in
  DoubleRow k-pair-interleaved [p=c-pair, (pc, l, kt)] layout, the bf16
  quarter in [p=c, (cb, l)] layout. Device DMAs are plain contiguous
  [128, N] block loads -- zero PE transposes, zero DVE copies, and HBM
  traffic drops from 32 MB to ~10.5 MB per core.
- W_attn is pre-scaled (x256, halves fp8 subnormal loss; the tanh's
  scale=1/256 undoes it), pre-cast, and pre-transposed on host into the
  DR weight layout [p, (pc, kt, e)] plus the bf16 [p, (cb, e)] blocks.
- bias[e,b] = Wa_s @ s[b].T is computed exactly on host (f64) -- the
  d-blocks never ship to the device.
- The masked-W_v tiles (column b holds W_v, zeros elsewhere) are built
  on host; the W_v contraction lands in PSUM row b for batch b, with
  the four e-blocks col-packed at PSUM partitions {0,32,64,96}+[0,8).
- The host gather sums the four col-group blocks and applies the
  softmax (cheap [8,1024] numpy), as before.

Device steady state per (lc, b) unit: 8 bf16 MMs + 12 fp8 DoubleRow MMs
(K=256 each) + 4 masked-W_v MMs, all N=512, with the deferred-wv MMs
used as LDWEIGHTS-hiding filler inside the next unit's MM stream; tanh
(+exact bias) on ACT in parallel. Everything is prefetched: all 32 enc
block DMAs are issued up front (SWDGE ring carries the fp8 halves,
HWDGE the consts + bf16 halves), so after a ~3us ramp the PE never
waits. A short dependency-free garbage-transpose burst covers the ramp
so the PE p-state is hot when the first real data lands.
"""

import numpy as np
import ml_dtypes

import concourse.bass as bass
import concourse.mybir as mybir
from concourse import bacc
from concourse.bass_utils import run_bass_kernel_spmd
from concourse.tile import TileContext

F32 = mybir.dt.float32
BF16 = mybir.dt.bfloat16
FP8 = mybir.dt.float8e4
AF = mybir.ActivationFunctionType
F8NP = ml_dtypes.float8_e4m3
BF16NP = ml_dtypes.bfloat16

L = 1024          # enc length
B = 64            # global batch
BL = 8            # batch per core
C = 1024          # enc feature dim (2*enc_hid)
D = 512           # dec feature dim
E = 512           # engry dim
NCORES = 8

NEB = E // 128    # 4 e-blocks
LCH = 512         # l-chunk processed per unit
NLC = L // LCH    # 2 chunks

# fp8 split: c < C8 runs in fp8e4 DoubleRow (2 c-blocks per matmul),
# c in [C8, C) stays bf16. W is pre-scaled by WSCALE before the fp8
# cast; the tanh activation's scale undoes it.
NC8 = 6           # fp8 c-blocks
NC16 = C // 128 - NC8  # bf16 c-blocks (2)
WSCALE = 256.0
C8 = NC8 * 128    # fp8 c-range (768)
NPC = NC8 // 2    # 256-c pair-chunks (3)


def build_nc():
    nc = bacc.Bacc("TRN2", target_bir_lowering=False, debug=False)

    enc8 = nc.dram_tensor("enc8", [NLC, BL, 128, NPC * 2 * LCH], FP8,
                          kind="ExternalInput").ap()
    enc16 = nc.dram_tensor("enc16", [NLC, BL, 128, NC16 * LCH], BF16,
                           kind="ExternalInput").ap()
    waT8p_d = nc.dram_tensor("waT8p", [128, NPC * 2 * E], FP8,
                             kind="ExternalInput").ap()
    waT16_d = nc.dram_tensor("waT16", [128, NC16 * E], BF16,
                             kind="ExternalInput").ap()
    bias_d = nc.dram_tensor("bias", [128, NEB * BL], F32,
                            kind="ExternalInput").ap()
    wvm_d = nc.dram_tensor("wv_mask", [128, NEB * BL * BL], BF16,
                           kind="ExternalInput").ap()
    # Blocked attention logits: per l-chunk, the four eb col-group blocks
    # live at PSUM partition rows {0,32,64,96}+[0,8). The host sums the
    # blocks and applies the softmax.
    out = nc.dram_tensor("out", [NLC, 128, LCH], F32, kind="ExternalOutput").ap()

    with TileContext(nc) as tc:
        with (
            tc.tile_pool(name="consts", bufs=1) as consts,
            tc.tile_pool(name="e8p", bufs=NLC * BL) as e8_pool,
            tc.tile_pool(name="e16p", bufs=NLC * BL) as e16_pool,
            tc.tile_pool(name="engry", bufs=2) as engry_pool,
            tc.tile_pool(name="pre", bufs=6, space="PSUM") as pre_pool,
            tc.tile_pool(name="att", bufs=2, space="PSUM") as att_pool,
        ):
            # p-state warmup: dependency-free garbage transposes keep the
            # PE pipe hot while the first DMAs land (output never read).
            # The warm tile rides the "pre" tag (PSUM is exactly full with
            # 6 pre banks + 2 att banks).
            garbage = consts.tile([128, 128], BF16, tag="garbage")
            nc.vector.memset(garbage[:], 0.0)
            warm_ps = pre_pool.tile([128, 512], BF16, tag="pre")
            for i in range(48):
                nc.tensor.transpose(
                    warm_ps[:, (i % 4) * 128:(i % 4) * 128 + 128],
                    garbage[:], garbage[:])

            # Consts ride HWDGE; the first unit's bf16 half goes right
            # after waT16 so the opening bf16 round isn't starved. The
            # SWDGE ring carries only the fp8 enc halves, in unit order.
            waT16 = consts.tile([128, NC16 * E], BF16, tag="waT16")
            nc.sync.dma_start(out=waT16[:], in_=waT16_d[:, :])

            e8_t, e16_t = {}, {}

            def fetch8(lc, b):
                t8 = e8_pool.tile([128, NPC * 2 * LCH], FP8, tag="e8",
                                  name=f"e8_{lc}_{b}")
                nc.gpsimd.dma_start(out=t8[:], in_=enc8[lc, b])
                e8_t[(lc, b)] = t8

            def fetch16(lc, b):
                t16 = e16_pool.tile([128, NC16 * LCH], BF16, tag="e16",
                                    name=f"e16_{lc}_{b}")
                nc.sync.dma_start(out=t16[:], in_=enc16[lc, b])
                e16_t[(lc, b)] = t16

            fetch8(0, 0)
            fetch16(0, 0)

            waT8p = consts.tile([128, NPC * 2 * E], FP8, tag="waT8p")
            nc.sync.dma_start(out=waT8p[:], in_=waT8p_d[:, :])
            bias_sbuf = consts.tile([128, NEB * BL], F32, tag="bias")
            nc.sync.dma_start(out=bias_sbuf[:], in_=bias_d[:, :])
            wv_mask = consts.tile([128, NEB * BL * BL], BF16, tag="wvm")
            nc.sync.dma_start(out=wv_mask[:], in_=wvm_d[:, :])

            for lc in range(NLC):
                for b in range(BL):
                    if (lc, b) != (0, 0):
                        fetch8(lc, b)
                        fetch16(lc, b)

            waT8v = waT8p.rearrange("p (pc two e) -> p pc two e",
                                    pc=NPC, two=2)

            # ---------------- main loop ----------------
            # att accumulation: eb's result lands in PSUM partitions
            # [32eb, 32eb+8), accumulated over b.
            #
            # PSUM-drain hiding: consecutive matmuls that accumulate into
            # the SAME PSUM bank serialize on the ~200-400ns result drain,
            # so the five c-chunk matmuls of each e-block are emitted
            # round-robin ACROSS the four e-blocks (4 rotating pre banks):
            # each matmul's drain hides under the next three banks'
            # streams. Each b's four wv matmuls are deferred into the next
            # b and emitted as ONE burst: their col_grp quadrants execute
            # concurrently, so the burst costs about a single N=512 slot.
            SEQ = [("b16", 0), ("dr", 0), ("b16", 1), ("dr", 1), ("dr", 2)]
            for lc in range(NLC):
                att_ps = att_pool.tile([128, LCH], F32, tag="att")

                def emit_wv(b, engries):
                    for eb in range(NEB):
                        nc.tensor.matmul(
                            att_ps[32 * eb:32 * eb + BL, :],
                            lhsT=wv_mask[:, eb * BL * BL + b * BL:
                                         eb * BL * BL + (b + 1) * BL],
                            rhs=engries[eb][:],
                            start=(b == 0),
                            stop=(b == BL - 1),
                            tile_position=(0, 32 * eb),
                        )

                pending = None
                for b in range(BL):
                    e8v = e8_t[(lc, b)].rearrange(
                        "p (pc l two) -> p pc two l", pc=NPC, two=2)
                    e16 = e16_t[(lc, b)]
                    pres = [pre_pool.tile([128, LCH], F32, tag="pre",
                                          name=f"pre{eb}_{lc}_{b}")
                            for eb in range(NEB)]
                    engries = []
                    for j, (kind, idx) in enumerate(SEQ):
                        for eb in range(NEB):
                            if kind == "b16":
                                nc.tensor.matmul(
                                    pres[eb][:],
                                    lhsT=waT16[:, idx * E + eb * 128:
                                               idx * E + (eb + 1) * 128],
                                    rhs=e16[:, idx * LCH:(idx + 1) * LCH],
                                    start=(j == 0),
                                    stop=(j == len(SEQ) - 1),
                                )
                            else:
                                nc.tensor.matmul(
                                    pres[eb][:],
                                    lhsT=waT8v[:, idx, :, eb * 128:(eb + 1) * 128],
                                    rhs=e8v[:, idx],
                                    start=(j == 0),
                                    stop=(j == len(SEQ) - 1),
                                    perf_mode=mybir.MatmulPerfMode.DoubleRow,
                                )
                            if j == len(SEQ) - 1:
                                engry = engry_pool.tile(
                                    [128, LCH], BF16, tag=f"engry{eb}",
                                    name=f"engry{eb}_{lc}_{b}")
                                nc.scalar.activation(
                                    engry[:], pres[eb][:], AF.Tanh,
                                    bias=bias_sbuf[:, eb * BL + b:
                                                   eb * BL + b + 1],
                                    scale=1.0 / WSCALE,
                                )
                                engries.append(engry)
                        if j == 0 and pending is not None:
                            emit_wv(*pending)
                    pending = (b, engries)
                # flush the last b's wv matmuls, then ship the blocked
                # logits.
                emit_wv(*pending)
                att_cp = consts.tile([128, LCH], F32, tag="att_cp",
                                     name=f"att_cp{lc}")
                nc.vector.tensor_copy(att_cp[:], att_ps[:])
                nc.sync.dma_start(out=out[lc], in_=att_cp[:])

    nc.compile()
    return nc


_NC_CACHE = None


def _get_nc():
    global _NC_CACHE
    if _NC_CACHE is None:
        _NC_CACHE = build_nc()
    return _NC_CACHE


def make_in_maps(enc_output, s, W_attn, W_v):
    enc = np.asarray(enc_output, dtype=np.float32)   # [L, B, C]
    s = np.asarray(s, dtype=np.float32)              # [1, B, D]
    W = np.asarray(W_attn, dtype=np.float32)         # [E, C+D]
    wv = np.asarray(W_v, dtype=np.float32)           # [1, E]

    # enc fp8 half -> DoubleRow k-pair image [lc, b, p, (pc, l, kt)]
    # with c = pc*256 + 2p + kt.
    e8 = enc[:, :, :C8].astype(F8NP)                 # [L, B, C8]
    e8 = e8.view(np.uint8).reshape(NLC, LCH, B, NPC, 128, 2)
    e8 = np.ascontiguousarray(e8.transpose(0, 2, 4, 3, 1, 5))
    e8 = e8.reshape(NLC, B, 128, NPC * LCH * 2).view(F8NP)

    # enc bf16 quarter -> [lc, b, p, (cb, l)] with c = C8 + cb*128 + p.
    e16 = enc[:, :, C8:].astype(BF16NP)              # [L, B, C-C8]
    e16 = e16.reshape(NLC, LCH, B, NC16, 128)
    e16 = np.ascontiguousarray(e16.transpose(0, 2, 4, 3, 1))
    e16 = e16.reshape(NLC, B, 128, NC16 * LCH)

    # DR weights [p, (pc, kt, e)] = fp8(WSCALE * W[e, pc*256 + 2p + kt])
    w8 = (W[:, :C8] * WSCALE).astype(F8NP)           # [E, C8]
    w8 = w8.reshape(E, NPC, 128, 2)                  # [e, pc, p, kt]
    waT8p = np.ascontiguousarray(w8.transpose(2, 1, 3, 0)).reshape(
        128, NPC * 2 * E)

    # bf16 weights [p, (cb, e)] = bf16(WSCALE * W[e, C8 + cb*128 + p])
    w16 = (W[:, C8:C] * WSCALE).astype(BF16NP)       # [E, NC16*128]
    w16 = w16.reshape(E, NC16, 128)
    waT16 = np.ascontiguousarray(w16.transpose(2, 1, 0)).reshape(
        128, NC16 * E)

    # exact bias[e, b] = Wa_s @ s[b].T in f64
    bias_full = np.einsum(
        'ed,bd->eb', W[:, C:].astype(np.float64),
        s[0].astype(np.float64)).astype(np.float32)  # [E, B]

    # masked W_v: [p, (eb, b, j)], column j==b holds wv[eb*128+p]
    wvm = np.zeros((128, NEB, BL, BL), np.float32)
    wvr = wv[0].reshape(NEB, 128)                    # [eb, p]
    for b in range(BL):
        wvm[:, :, b, b] = wvr.T
    wv_mask = np.ascontiguousarray(
        wvm.astype(BF16NP).reshape(128, NEB * BL * BL))

    in_maps = []
    for i in range(NCORES):
        bias = np.ascontiguousarray(
            bias_full[:, i * BL:(i + 1) * BL].reshape(NEB, 128, BL)
            .transpose(1, 0, 2).reshape(128, NEB * BL))
        in_maps.append({
            "enc8": np.ascontiguousarray(e8[:, i * BL:(i + 1) * BL]),
            "enc16": np.ascontiguousarray(e16[:, i * BL:(i + 1) * BL]),
            "waT8p": waT8p,
            "waT16": waT16,
            "bias": bias,
            "wv_mask": wv_mask,
        })
    return in_maps


def kernel(enc_output, s, W_attn, W_v):
    nc = _get_nc()
    in_maps = make_in_maps(enc_output, s, W_attn, W_v)
    res = run_bass_kernel_spmd(nc, in_maps, core_ids=list(range(NCORES)))
    outs = []
    for i in range(NCORES):
        blk = res.results[i]["out"]  # [NLC, 128, LCH] blocked logits
        att = sum(blk[:, 32 * g:32 * g + BL, :] for g in range(4))  # [NLC, BL, LCH]
        att = np.concatenate([att[lc] for lc in range(NLC)], axis=1)  # [BL, L]
        m = att.max(axis=1, keepdims=True)
        e = np.exp(att - m)
        outs.append((e / e.sum(axis=1, keepdims=True)).astype(np.float32))
    return np.concatenate(outs, axis=0)


# revision 11
# speedup vs baseline: 1.3443x; 1.1186x over previous
"""Trainium2 Bass kernel for nn_Attention_19739669692939 (sparse_attention).

Reference computation (shapes: L=1024, B=64, C=1024, D=512, E=512):
    Wa_e = W_attn[:, :C]        # [E, C]
    Wa_s = W_attn[:, C:]        # [E, D]
    pre  = enc_output @ Wa_e.T + s @ Wa_s.T     # [L, B, E] (s broadcast over L)
    engry = tanh(pre)
    att[b, l] = engry[l, b, :] @ W_v[0, :]
    out = softmax(att, axis=-1)                 # [B, 1024]

Distribution: pure data-parallel over batch. Core i handles batches
[8i, 8i+8); no collectives.

v2 design: ALL data-layout work happens on the host before staging to
HBM, so the device runs matmuls only:
- enc is pre-cast on host (fp8e4m3 for c<768, bf16 for c>=768) and
  pre-arranged in the exact SBUF image the PE consumes: the fp8 half in
  DoubleRow k-pair-interleaved [p=c-pair, (pc, l, kt)] layout, the bf16
  quarter in [p=c, (cb, l)] layout. Device DMAs are plain contiguous
  [128, N] block loads -- zero PE transposes, zero DVE copies, and HBM
  traffic drops from 32 MB to ~10.5 MB per core.
- W_attn is pre-scaled (x256, halves fp8 subnormal loss; the tanh's
  scale=1/256 undoes it), pre-cast, and pre-transposed on host into the
  DR weight layout [p, (pc, kt, e)] plus the bf16 [p, (cb, e)] blocks.
- bias[e,b] = Wa_s @ s[b].T is computed exactly on host (f64) -- the
  d-blocks never ship to the device.
- The masked-W_v tiles (column b holds W_v, zeros elsewhere) are built
  on host; the W_v contraction lands in PSUM row b for batch b, with
  the four e-blocks col-packed at PSUM partitions {0,32,64,96}+[0,8).
- The host gather sums the four col-group blocks and applies the
  softmax (cheap [8,1024] numpy), as before.

Device steady state per (lc, b) unit: 8 bf16 MMs + 12 fp8 DoubleRow MMs
(K=256 each) + 4 masked-W_v MMs, all N=512, with the deferred-wv MMs
used as LDWEIGHTS-hiding filler inside the next unit's MM stream; tanh
(+exact bias) on ACT in parallel. Everything is prefetched: all 32 enc
block DMAs are issued up front (SWDGE ring carries the fp8 halves,
HWDGE the consts + bf16 halves), so after a ~3us ramp the PE never
waits. A short dependency-free garbage-transpose burst covers the ramp
so the PE p-state is hot when the first real data lands.
"""

import numpy as np
import ml_dtypes

import concourse.bass as bass
import concourse.mybir as mybir
from concourse import bacc
from concourse.bass_utils import run_bass_kernel_spmd
from concourse.tile import TileContext

F32 = mybir.dt.float32
BF16 = mybir.dt.bfloat16
FP8 = mybir.dt.float8e4
AF = mybir.ActivationFunctionType
F8NP = ml_dtypes.float8_e4m3
BF16NP = ml_dtypes.bfloat16

L = 1024          # enc length
B = 64            # global batch
BL = 8            # batch per core
C = 1024          # enc feature dim (2*enc_hid)
D = 512           # dec feature dim
E = 512           # engry dim
NCORES = 8

NEB = E // 128    # 4 e-blocks
LCH = 512         # l-chunk processed per unit
NLC = L // LCH    # 2 chunks

# fp8 split: c < C8 runs in fp8e4 DoubleRow (2 c-blocks per matmul),
# c in [C8, C) stays bf16. W is pre-scaled by WSCALE before the fp8
# cast; the tanh activation's scale undoes it.
NC8 = 6           # fp8 c-blocks
NC16 = C // 128 - NC8  # bf16 c-blocks (2)
WSCALE = 256.0
C8 = NC8 * 128    # fp8 c-range (768)
NPC = NC8 // 2    # 256-c pair-chunks (3)


def build_nc():
    nc = bacc.Bacc("TRN2", target_bir_lowering=False, debug=False)

    enc8 = nc.dram_tensor("enc8", [NLC, BL, 128, NPC * 2 * LCH], FP8,
                          kind="ExternalInput").ap()
    enc16 = nc.dram_tensor("enc16", [NLC, BL, 128, NC16 * LCH], BF16,
                           kind="ExternalInput").ap()
    waT8p_d = nc.dram_tensor("waT8p", [128, NPC * 2 * E], FP8,
                             kind="ExternalInput").ap()
    waT16_d = nc.dram_tensor("waT16", [128, NC16 * E], BF16,
                             kind="ExternalInput").ap()
    bias_d = nc.dram_tensor("bias", [128, NEB * BL], F32,
                            kind="ExternalInput").ap()
    wvT_d = nc.dram_tensor("wvT", [128, NEB], F32,
                           kind="ExternalInput").ap()
    ones_d = nc.dram_tensor("ones_mask", [128, BL * BL], BF16,
                            kind="ExternalInput").ap()
    # Attention logits, row b = batch b; host applies the softmax.
    out = nc.dram_tensor("out", [NLC, BL, LCH], F32, kind="ExternalOutput").ap()

    with TileContext(nc) as tc:
        with (
            tc.tile_pool(name="consts", bufs=1) as consts,
            tc.tile_pool(name="e8p", bufs=NLC * BL) as e8_pool,
            tc.tile_pool(name="e16p", bufs=NLC * BL) as e16_pool,
            tc.tile_pool(name="engry", bufs=2) as engry_pool,
            tc.tile_pool(name="z", bufs=2) as z_pool,
            tc.tile_pool(name="pre", bufs=6, space="PSUM") as pre_pool,
            tc.tile_pool(name="att", bufs=2, space="PSUM") as att_pool,
        ):
            # p-state warmup: dependency-free garbage transposes keep the
            # PE pipe hot while the first DMAs land (output never read).
            # The warm tile rides the "pre" tag (PSUM is exactly full with
            # 6 pre banks + 2 att banks).
            garbage = consts.tile([128, 128], BF16, tag="garbage")
            nc.vector.memset(garbage[:], 0.0)
            warm_ps = pre_pool.tile([128, 512], BF16, tag="pre")
            for i in range(48):
                nc.tensor.transpose(
                    warm_ps[:, (i % 4) * 128:(i % 4) * 128 + 128],
                    garbage[:], garbage[:])

            # Consts ride HWDGE; the first unit's bf16 half goes right
            # after waT16 so the opening bf16 round isn't starved. The
            # SWDGE ring carries only the fp8 enc halves, in unit order.
            waT16 = consts.tile([128, NC16 * E], BF16, tag="waT16")
            nc.sync.dma_start(out=waT16[:], in_=waT16_d[:, :])

            e8_t, e16_t = {}, {}

            def fetch8(lc, b):
                t8 = e8_pool.tile([128, NPC * 2 * LCH], FP8, tag="e8",
                                  name=f"e8_{lc}_{b}")
                nc.gpsimd.dma_start(out=t8[:], in_=enc8[lc, b])
                e8_t[(lc, b)] = t8

            def fetch16(lc, b):
                t16 = e16_pool.tile([128, NC16 * LCH], BF16, tag="e16",
                                    name=f"e16_{lc}_{b}")
                nc.sync.dma_start(out=t16[:], in_=enc16[lc, b])
                e16_t[(lc, b)] = t16

            fetch8(0, 0)
            fetch16(0, 0)

            waT8p = consts.tile([128, NPC * 2 * E], FP8, tag="waT8p")
            nc.sync.dma_start(out=waT8p[:], in_=waT8p_d[:, :])
            bias_sbuf = consts.tile([128, NEB * BL], F32, tag="bias")
            nc.sync.dma_start(out=bias_sbuf[:], in_=bias_d[:, :])
            wvT = consts.tile([128, NEB], F32, tag="wvT")
            nc.sync.dma_start(out=wvT[:], in_=wvT_d[:, :])
            ones_mask = consts.tile([128, BL * BL], BF16, tag="ones")
            nc.sync.dma_start(out=ones_mask[:], in_=ones_d[:, :])

            for lc in range(NLC):
                for b in range(BL):
                    if (lc, b) != (0, 0):
                        fetch8(lc, b)
                        fetch16(lc, b)

            waT8v = waT8p.rearrange("p (pc two e) -> p pc two e",
                                    pc=NPC, two=2)

            # ---------------- main loop ----------------
            # PSUM-drain hiding: consecutive matmuls that accumulate into
            # the SAME PSUM bank serialize on the ~200-400ns result drain,
            # so the five c-chunk matmuls of each e-block are emitted
            # round-robin ACROSS the four e-blocks (4 rotating pre banks):
            # each matmul's drain hides under the next three banks'
            # streams.
            #
            # W_v contraction: the per-partition weighting runs on the
            # (otherwise idle) DVE as a chain of 4 scalar_tensor_tensor
            # ops, z[p,l] = sum_eb wvT[p,eb]*engry[eb][p,l], with the
            # final op casting to bf16. The remaining partition reduction
            # is ONE ones-mask matmul per b (vs 4 masked-W_v matmuls):
            # column b of ones_mask is all-ones, so batch b's logits land
            # in PSUM row b, accumulated over the b-group. The matmul is
            # deferred into the next b's stream.
            SEQ = [("b16", 0), ("dr", 0), ("b16", 1), ("dr", 1), ("dr", 2)]
            for lc in range(NLC):
                att_ps = att_pool.tile([128, LCH], F32, tag="att")

                def emit_att(b, z_out):
                    nc.tensor.matmul(
                        att_ps[0:BL, :],
                        lhsT=ones_mask[:, b * BL:(b + 1) * BL],
                        rhs=z_out[:],
                        start=(b == 0),
                        stop=(b == BL - 1),
                        tile_position=(0, 0),
                    )

                pending = None
                for b in range(BL):
                    e8v = e8_t[(lc, b)].rearrange(
                        "p (pc l two) -> p pc two l", pc=NPC, two=2)
                    e16 = e16_t[(lc, b)]
                    pres = [pre_pool.tile([128, LCH], F32, tag="pre",
                                          name=f"pre{eb}_{lc}_{b}")
                            for eb in range(NEB)]
                    engries = []
                    for j, (kind, idx) in enumerate(SEQ):
                        for eb in range(NEB):
                            if kind == "b16":
                                nc.tensor.matmul(
                                    pres[eb][:],
                                    lhsT=waT16[:, idx * E + eb * 128:
                                               idx * E + (eb + 1) * 128],
                                    rhs=e16[:, idx * LCH:(idx + 1) * LCH],
                                    start=(j == 0),
                                    stop=(j == len(SEQ) - 1),
                                )
                            else:
                                nc.tensor.matmul(
                                    pres[eb][:],
                                    lhsT=waT8v[:, idx, :, eb * 128:(eb + 1) * 128],
                                    rhs=e8v[:, idx],
                                    start=(j == 0),
                                    stop=(j == len(SEQ) - 1),
                                    perf_mode=mybir.MatmulPerfMode.DoubleRow,
                                )
                            if j == len(SEQ) - 1:
                                engry = engry_pool.tile(
                                    [128, LCH], BF16, tag=f"engry{eb}",
                                    name=f"engry{eb}_{lc}_{b}")
                                nc.scalar.activation(
                                    engry[:], pres[eb][:], AF.Tanh,
                                    bias=bias_sbuf[:, eb * BL + b:
                                                   eb * BL + b + 1],
                                    scale=1.0 / WSCALE,
                                )
                                engries.append(engry)
                        if j == 0 and pending is not None:
                            emit_att(*pending)
                    # DVE: z = sum_eb wvT[:,eb] * engry[eb], final op
                    # casts to bf16 for the cheap bf16 partition-reduce MM.
                    z_acc = z_pool.tile([128, LCH], F32, tag="zacc",
                                        name=f"zacc_{lc}_{b}")
                    nc.vector.scalar_tensor_tensor(
                        out=z_acc[:], in0=engries[0][:], scalar=wvT[:, 0:1],
                        in1=engries[0][:],
                        op0=mybir.AluOpType.mult, op1=mybir.AluOpType.bypass)
                    for eb in range(1, NEB - 1):
                        nc.vector.scalar_tensor_tensor(
                            out=z_acc[:], in0=engries[eb][:],
                            scalar=wvT[:, eb:eb + 1], in1=z_acc[:],
                            op0=mybir.AluOpType.mult, op1=mybir.AluOpType.add)
                    z_out = z_pool.tile([128, LCH], BF16, tag="zout",
                                        name=f"zout_{lc}_{b}")
                    nc.vector.scalar_tensor_tensor(
                        out=z_out[:], in0=engries[NEB - 1][:],
                        scalar=wvT[:, NEB - 1:NEB], in1=z_acc[:],
                        op0=mybir.AluOpType.mult, op1=mybir.AluOpType.add)
                    pending = (b, z_out)
                # flush the last b's logits matmul, then ship row-packed
                # logits [BL, LCH].
                emit_att(*pending)
                att_cp = consts.tile([BL, LCH], F32, tag="att_cp",
                                     name=f"att_cp{lc}")
                nc.vector.tensor_copy(att_cp[:], att_ps[0:BL, :])
                nc.sync.dma_start(out=out[lc], in_=att_cp[:])

    nc.compile()
    return nc


_NC_CACHE = None


def _get_nc():
    global _NC_CACHE
    if _NC_CACHE is None:
        _NC_CACHE = build_nc()
    return _NC_CACHE


def make_in_maps(enc_output, s, W_attn, W_v):
    enc = np.asarray(enc_output, dtype=np.float32)   # [L, B, C]
    s = np.asarray(s, dtype=np.float32)              # [1, B, D]
    W = np.asarray(W_attn, dtype=np.float32)         # [E, C+D]
    wv = np.asarray(W_v, dtype=np.float32)           # [1, E]

    # enc fp8 half -> DoubleRow k-pair image [lc, b, p, (pc, l, kt)]
    # with c = pc*256 + 2p + kt.
    e8 = enc[:, :, :C8].astype(F8NP)                 # [L, B, C8]
    e8 = e8.view(np.uint8).reshape(NLC, LCH, B, NPC, 128, 2)
    e8 = np.ascontiguousarray(e8.transpose(0, 2, 4, 3, 1, 5))
    e8 = e8.reshape(NLC, B, 128, NPC * LCH * 2).view(F8NP)

    # enc bf16 quarter -> [lc, b, p, (cb, l)] with c = C8 + cb*128 + p.
    e16 = enc[:, :, C8:].astype(BF16NP)              # [L, B, C-C8]
    e16 = e16.reshape(NLC, LCH, B, NC16, 128)
    e16 = np.ascontiguousarray(e16.transpose(0, 2, 4, 3, 1))
    e16 = e16.reshape(NLC, B, 128, NC16 * LCH)

    # DR weights [p, (pc, kt, e)] = fp8(WSCALE * W[e, pc*256 + 2p + kt])
    w8 = (W[:, :C8] * WSCALE).astype(F8NP)           # [E, C8]
    w8 = w8.reshape(E, NPC, 128, 2)                  # [e, pc, p, kt]
    waT8p = np.ascontiguousarray(w8.transpose(2, 1, 3, 0)).reshape(
        128, NPC * 2 * E)

    # bf16 weights [p, (cb, e)] = bf16(WSCALE * W[e, C8 + cb*128 + p])
    w16 = (W[:, C8:C] * WSCALE).astype(BF16NP)       # [E, NC16*128]
    w16 = w16.reshape(E, NC16, 128)
    waT16 = np.ascontiguousarray(w16.transpose(2, 1, 0)).reshape(
        128, NC16 * E)

    # exact bias[e, b] = Wa_s @ s[b].T in f64
    bias_full = np.einsum(
        'ed,bd->eb', W[:, C:].astype(np.float64),
        s[0].astype(np.float64)).astype(np.float32)  # [E, B]

    # per-partition W_v columns [p, eb] and the ones mask (column j==b
    # is all-ones) for the partition-reduce matmul
    wvT = np.ascontiguousarray(wv[0].reshape(NEB, 128).T.astype(np.float32))
    ones_mask = np.zeros((128, BL, BL), np.float32)
    for b in range(BL):
        ones_mask[:, b, b] = 1.0
    ones_mask = np.ascontiguousarray(
        ones_mask.astype(BF16NP).reshape(128, BL * BL))

    in_maps = []
    for i in range(NCORES):
        bias = np.ascontiguousarray(
            bias_full[:, i * BL:(i + 1) * BL].reshape(NEB, 128, BL)
            .transpose(1, 0, 2).reshape(128, NEB * BL))
        in_maps.append({
            "enc8": np.ascontiguousarray(e8[:, i * BL:(i + 1) * BL]),
            "enc16": np.ascontiguousarray(e16[:, i * BL:(i + 1) * BL]),
            "waT8p": waT8p,
            "waT16": waT16,
            "bias": bias,
            "wvT": wvT,
            "ones_mask": ones_mask,
        })
    return in_maps


def kernel(enc_output, s, W_attn, W_v):
    nc = _get_nc()
    in_maps = make_in_maps(enc_output, s, W_attn, W_v)
    res = run_bass_kernel_spmd(nc, in_maps, core_ids=list(range(NCORES)))
    outs = []
    for i in range(NCORES):
        blk = res.results[i]["out"]  # [NLC, BL, LCH] logits
        att = np.concatenate([blk[lc] for lc in range(NLC)], axis=1)  # [BL, L]
        m = att.max(axis=1, keepdims=True)
        e = np.exp(att - m)
        outs.append((e / e.sum(axis=1, keepdims=True)).astype(np.float32))
    return np.concatenate(outs, axis=0)
